# revision 29
# baseline (speedup 1.0000x reference)
"""Trainium2 kernel for nn_EuclideanEmbedding (edge-scale + segment_sum).

Computes: out[n, :] = inv * sum_{e: receivers[e]==n} sh_vectors[e, :] * cutoffs[e]

Distribution (host side, inside kernel()):
  - Edges sharded across the 8 NeuronCores BY RECEIVER NODE RANGE: core c owns
    nodes [c*6250, (c+1)*6250) and exactly the edges targeting them.  Each core
    produces a disjoint output slice, so no collective is needed.

Device layout (v2 — bf16 + PE-array segmented reduction):
  - Nodes are degree-sorted (desc) and packed into capacity groups.  A group
    with slot capacity c stacks k = floor(128/c) node-columns on the partition
    axis: partition p = n_lo*c + s  (n_lo in [0,k), slot s in [0,c)).
    Free axis per node-column block of 8 nodes: col = 128*i + d*8 + n8.
  - sh and cutoffs are converted to bf16 on the host (harness gate is
    rel_err < 2e-2; bf16 keeps us ~5e-3).  This halves HBM traffic — the
    fp32 baseline was HBM-bound at ~64us of DMA window.
  - The elementwise multiply (sh * cut) runs as bf16 TensorTensor ops split
    between the Vector and GpSimd engines (bf16 packed innermost -> DVE 2x
    mode).  The cut operand broadcasts over d via a zero-stride AP dim.
  - The segmented reduction runs on the OTHERWISE-IDLE PE array: one matmul
    per 8-node block with lhsT (stationary) = scaled data [kc, 128] and
    rhs (moving) = a static ones-block [kc, k] (ones[p, j] = (p // c == j)).
    out[d*8+n8, j] = sum_s scl[(j,s), d*8+n8] lands in PSUM fp32.  This
    removes the ~30us dtype-independent DVE tensor_reduce of the baseline.
  - PSUM eviction via the Scalar (ACT) engine activation-copy, which also
    applies the inv_avg_num_neighbors scale for free.
  - Output written per chunk (overlapped), not as one serial tail DMA.
"""

import os

import ml_dtypes
import numpy as np

# ---------------------------------------------------------------- constants
N_NODES = 50_000
D_SH = 16
N_CORES = 8
NPC = N_NODES // N_CORES          # 6250 nodes per core
NB_MAX = 56                       # max 8-node blocks per chunk (DMA/psum unit)
NB_PIECE = 14                     # blocks per Vector-multiply piece
# The multiply runs entirely on the Vector engine (bf16 2x mode, ~0.62 ns/col
# measured).  GpSimd/Pool measured 2.5-4.9 ns/col on this AP shape and its
# slow ops became the pipeline critical path, so it gets no share.

_NC_CACHE: dict = {}
LAST_RESULTS = None  # BassKernelResults of the most recent run (for test.py)


# ---------------------------------------------------------------- geometry
def plan_chunks(dmax: np.ndarray):
    """Greedy capacity grouping from the SPMD-uniform per-position max degree
    (descending).  Returns a list of chunk dicts with all offsets."""
    q, groups = 0, []
    npos = dmax.shape[0]
    while q < npos:
        d0 = max(int(dmax[q]), 1)
        assert d0 <= 128, f"node degree {d0} > 128 unsupported by this layout"
        k = max(1, 128 // d0)
        q2 = q
        while q2 < npos and max(1, 128 // max(int(dmax[q2]), 1)) == k:
            q2 += 1
        n = -(-(q2 - q) // (8 * k)) * (8 * k)
        n = min(n, -(-(npos - q) // (8 * k)) * (8 * k))
        c = int(dmax[q:min(q + n, npos)].max())
        groups.append((k, max(c, 1), n))
        q += n

    chunks = []
    node_start = sh_off = cut_off = out_off = ones_off = 0
    for k, c, n in groups:
        nb_total = n // (8 * k)
        done = 0
        while done < nb_total:
            nb = min(NB_MAX, nb_total - done)
            kc = k * c
            F = 128 * nb
            # per-partition row layout: [0, F) sh data, [F, F+8nb) cutoffs —
            # one dma_start moves both, keeping total input DMAs within the
            # 8 HWDGE semaphores (no mid-stream semaphore recycling)
            L = F + 8 * nb
            chunks.append(dict(k=k, c=c, kc=kc, nb=nb, F=F, L=L,
                               node_start=node_start, sh_off=sh_off,
                               out_off=out_off, ones_off=ones_off))
            node_start += 8 * k * nb
            sh_off += kc * L
            out_off += k * nb
            ones_off += k
            done += nb
    tot = dict(nodes=node_start, sh=sh_off, out=out_off, ones=ones_off)
    # device-side processing order: ascending size (fast pipeline priming,
    # arrivals track the multiply), with the second-smallest moved last so
    # the drain tail is short; host layout offsets are order-independent.
    by_size = sorted(range(len(chunks)),
                     key=lambda i: chunks[i]["kc"] * chunks[i]["L"])
    sched = by_size[:1] + by_size[2:] + by_size[1:2]
    return chunks, tot, sched


# ---------------------------------------------------------------- device IR
def build_nc(key, chunks, tot, sched):
    if key in _NC_CACHE:
        return _NC_CACHE[key]

    import concourse.bacc as bacc
    import concourse.bass as bass
    import concourse.mybir as mybir
    from concourse import tile

    nc = bacc.Bacc("TRN2", target_bir_lowering=False, debug=False)

    sh = nc.dram_tensor("sh", [tot["sh"]], mybir.dt.bfloat16,
                        kind="ExternalInput")
    ones = nc.dram_tensor("ones", [128, tot["ones"]], mybir.dt.bfloat16,
                          kind="ExternalInput")
    out = nc.dram_tensor("out", [128, tot["out"]], mybir.dt.float32,
                         kind="ExternalOutput")

    with tile.TileContext(nc) as tc:
        # shp holds EVERY chunk simultaneously (~58KB/partition total): all
        # sh dma_starts then issue up-front with no WAR gating, which keeps
        # the descriptor ring loaded evenly across all 16 DMA engines.  A
        # late, lone dma_start lands on only 1-2 drained engines (~22 B/ns
        # each) and starves the pipeline.  Input DMAs stay within the 8
        # HWDGE semaphores (7 sh + cut slices + ones, only the tiny first
        # transfers get recycled); output DMAs go through GpSimd's SWDGE
        # lanes so they never steal an input semaphore.
        with (
            tc.tile_pool(name="const", bufs=1) as cpool,
            tc.tile_pool(name="shp", bufs=len(chunks)) as shp,
            tc.tile_pool(name="sclv", bufs=3) as sclvp,
            tc.tile_pool(name="osb", bufs=6) as osbp,
            tc.tile_pool(name="ps", bufs=4, space="PSUM") as psp,
        ):
            ones_t = cpool.tile([128, tot["ones"]], mybir.dt.bfloat16)

            for chi, ci in enumerate(sched):
                ch = chunks[ci]
                k, c, kc, nb, F, L = (ch["k"], ch["c"], ch["kc"], ch["nb"],
                                      ch["F"], ch["L"])
                sh_t = shp.tile([kc, L], mybir.dt.bfloat16, tag="sh")
                src = bass.AP(sh.ap().tensor, ch["sh_off"], [[L, kc], [1, L]])
                nc.sync.dma_start(sh_t[:], src)
                if chi == 0:
                    # ones is only needed by the first matmul; emitted
                    # behind the first (smallest) chunk
                    nc.sync.dma_start(ones_t[:], ones[:])

                # scl[p, i, d, n8] = sh[p, i, d, n8] * cut[p, 8*i + n8]
                # on Vector, in ~NB_PIECE-block pieces so PE starts on the
                # first piece while the rest are still multiplying
                scl_tiles = []
                for pi, b0 in enumerate(range(0, nb, NB_PIECE)):
                    b1 = min(nb, b0 + NB_PIECE)
                    nbe = b1 - b0
                    scl = sclvp.tile([kc, nbe * 128], mybir.dt.bfloat16,
                                     tag=f"scl{pi}")
                    pdim = list(sh_t[:].ap[0])
                    sh4 = bass.AP(sh_t[:].tensor, sh_t[:].offset + b0 * 128,
                                  [pdim, [128, nbe], [8, D_SH], [1, 8]])
                    scl4 = bass.AP(scl[:].tensor, scl[:].offset,
                                   [list(scl[:].ap[0]), [128, nbe],
                                    [8, D_SH], [1, 8]])
                    cut4 = bass.AP(sh_t[:].tensor,
                                   sh_t[:].offset + F + b0 * 8,
                                   [pdim, [8, nbe], [0, D_SH], [1, 8]])
                    nc.vector.tensor_mul(scl4, sh4, cut4)
                    scl_tiles.append((scl, b0, b1))

                # PE: per 8-node block, out[(d,n8), j] = sum_s scl[(j,s),...]
                ps_t = psp.tile([128, k * nb], mybir.dt.float32, tag="ps")
                ones_ap = bass.AP(ones_t[:].tensor,
                                  ones_t[:].offset + ch["ones_off"],
                                  [[ones_t[:].ap[0][0], kc], [1, k]])
                for scl, b0, b1 in scl_tiles:
                    for i in range(b0, b1):
                        lhsT = bass.AP(scl[:].tensor,
                                       scl[:].offset + (i - b0) * 128,
                                       [list(scl[:].ap[0]), [1, 128]])
                        nc.tensor.matmul(ps_t[:, i * k:(i + 1) * k],
                                         lhsT, ones_ap)

                # evict PSUM -> SBUF (inv is folded into the ones weights);
                # out DMA issued from the idle GpSimd engine: SWDGE has its
                # own semaphore lanes, so outputs never recycle one of the 8
                # HWDGE semaphores the input stream depends on
                osb = osbp.tile([128, k * nb], mybir.dt.float32, tag="osb")
                nc.scalar.copy(osb[:], ps_t[:])
                dst = bass.AP(out.ap().tensor, ch["out_off"],
                              [[tot["out"], 128], [1, k * nb]])
                nc.gpsimd.dma_start(dst, osb[:])

    nc.compile()
    _NC_CACHE[key] = nc
    return nc


# ---------------------------------------------------------------- host shard
def shard_inputs(sh_vectors, cutoffs, receivers, inv_avg_num_neighbors):
    sh_np = np.ascontiguousarray(np.asarray(sh_vectors, dtype=np.float32))
    cut_np = np.asarray(cutoffs, dtype=np.float32).ravel()
    rec = np.asarray(receivers).astype(np.int64).ravel()
    inv_val = np.float32(np.asarray(inv_avg_num_neighbors).ravel()[0])

    order = np.argsort(rec, kind="stable")
    rec_sorted = rec[order]
    first = np.searchsorted(rec_sorted, rec_sorted, side="left")
    occ = (np.arange(rec.size) - first).astype(np.int64)
    bounds = np.searchsorted(rec_sorted, np.arange(0, N_NODES + 1, NPC))

    deg = np.bincount(rec, minlength=N_NODES).reshape(N_CORES, NPC)
    node_orders, pos_of_node, dsort = [], [], np.empty((N_CORES, NPC), np.int64)
    for c in range(N_CORES):
        no = np.argsort(-deg[c], kind="stable")
        node_orders.append(no)
        pon = np.empty(NPC, dtype=np.int64)
        pon[no] = np.arange(NPC)
        pos_of_node.append(pon)
        dsort[c] = deg[c][no]
    dmax = dsort.max(axis=0)

    chunks, tot, sched = plan_chunks(dmax)

    # per-chunk lookup arrays indexed by chunk id
    g_start = np.array([ch["node_start"] for ch in chunks], np.int64)
    g_k = np.array([ch["k"] for ch in chunks], np.int64)
    g_c = np.array([ch["c"] for ch in chunks], np.int64)
    g_F = np.array([ch["F"] for ch in chunks], np.int64)
    g_L = np.array([ch["L"] for ch in chunks], np.int64)
    g_sh = np.array([ch["sh_off"] for ch in chunks], np.int64)
    g_out = np.array([ch["out_off"] for ch in chunks], np.int64)

    # static ones blocks with the inv scale folded into the weights (the PE
    # matmul then produces the final scaled sums directly)
    ones_dev = np.zeros((128, tot["ones"]), dtype=ml_dtypes.bfloat16)
    for ch in chunks:
        k, c = ch["k"], ch["c"]
        p = np.arange(k * c)
        ones_dev[p, ch["ones_off"] + p // c] = inv_val

    sh_bf = sh_np.astype(ml_dtypes.bfloat16)
    cut_bf = cut_np.astype(ml_dtypes.bfloat16)

    in_maps = []
    decode = []   # per core arrays to invert the layout on output
    for cid in range(N_CORES):
        lo, hi = bounds[cid], bounds[cid + 1]
        e = order[lo:hi]
        l = rec_sorted[lo:hi] - cid * NPC
        o = occ[lo:hi]
        q = pos_of_node[cid][l]

        gi = np.searchsorted(g_start, q, side="right") - 1
        k_e, c_e, F_e, L_e = g_k[gi], g_c[gi], g_F[gi], g_L[gi]
        dq = q - g_start[gi]
        jj = dq // k_e
        n_lo = dq - jj * k_e
        i_loc = jj >> 3
        n8 = jj & 7
        p = n_lo * c_e + o
        assert (o < c_e).all(), "slot overflow: degree exceeds group capacity"

        sh_dev = np.zeros(tot["sh"], dtype=ml_dtypes.bfloat16)
        base = g_sh[gi] + p * L_e + 128 * i_loc + n8
        shv = sh_bf[e]
        for d in range(D_SH):
            sh_dev[base + 8 * d] = shv[:, d]
        # cutoffs ride in the same per-chunk block, cols [F, F+8nb)
        sh_dev[g_sh[gi] + p * L_e + F_e + jj] = cut_bf[e]
        in_maps.append({"sh": sh_dev, "ones": ones_dev})

        # output decode for every position q in [0, NPC)
        qq = np.arange(NPC, dtype=np.int64)
        gq = np.searchsorted(g_start, qq, side="right") - 1
        dqq = qq - g_start[gq]
        jjq = dqq // g_k[gq]
        col = g_out[gq] + (jjq >> 3) * g_k[gq] + (dqq - jjq * g_k[gq])
        row8 = jjq & 7
        decode.append((col, row8))

    return in_maps, chunks, tot, sched, node_orders, decode


# ---------------------------------------------------------------- profiling
def _install_ntff_shim() -> bool:
    """This image's antenv lacks the axon_hooks shim that bass_utils imports
    for trace=True under axon.  Recreate it from trn_agent_boot's ctypes hook
    so NTFF profiling works.  Returns True on success."""
    try:
        import sys
        import types

        import antenv

        if getattr(antenv, "axon_hooks", None) is not None:
            return True
        import trn_agent_boot.trn_boot as tb

        hook = tb._ntff_profile_via_ctypes("/opt/axon/libaxon_pjrt.so")
        mod = types.ModuleType("antenv.axon_hooks")
        mod._hook = hook
        mod.get_axon_ntff_profile_hook = lambda: mod._hook
        mod.set_axon_ntff_profile_hook = lambda h: setattr(mod, "_hook", h)
        sys.modules["antenv.axon_hooks"] = mod
        antenv.axon_hooks = mod
        return hook is not None
    except Exception as e:  # profiling is best-effort; the run must not break
        print(f"ntff shim unavailable: {e!r}")
        return False


# ---------------------------------------------------------------- entrypoint
def kernel(sh_vectors, cutoffs, receivers, inv_avg_num_neighbors) -> np.ndarray:
    global LAST_RESULTS
    from concourse.bass_utils import run_bass_kernel_spmd

    in_maps, chunks, tot, sched, node_orders, decode = shard_inputs(
        sh_vectors, cutoffs, receivers, inv_avg_num_neighbors)
    key = tuple((ch["k"], ch["c"], ch["nb"]) for ch in chunks)
    nc = build_nc(key, chunks, tot, sched)

    trace = os.environ.get("KERNEL_TRACE", "0") == "1"
    if trace:
        trace = _install_ntff_shim()
    res = run_bass_kernel_spmd(nc, in_maps, core_ids=list(range(N_CORES)),
                               trace=trace)
    LAST_RESULTS = res

    full = np.empty((N_NODES, D_SH), dtype=np.float32)
    for cid in range(N_CORES):
        o = np.asarray(res.results[cid]["out"], dtype=np.float32)
        col, row8 = decode[cid]
        blk = np.empty((NPC, D_SH), dtype=np.float32)
        for d in range(D_SH):
            blk[:, d] = o[8 * d + row8, col]
        full[cid * NPC + node_orders[cid]] = blk
    return full


# revision 32
# speedup vs baseline: 1.6719x; 1.6719x over previous
"""Trainium2 kernel for nn_EuclideanEmbedding (edge-scale + segment_sum).

Computes: out[n, :] = inv * sum_{e: receivers[e]==n} sh_vectors[e, :] * cutoffs[e]

Distribution (host side, inside kernel()):
  - Edges sharded across the 8 NeuronCores BY RECEIVER NODE RANGE: core c owns
    nodes [c*6250, (c+1)*6250) and exactly the edges targeting them.  Each core
    produces a disjoint output slice, so no collective is needed.

Device design (bf16 + PE-array segmented reduction):
  - Nodes are degree-sorted (desc) and packed into capacity groups.  A group
    with slot capacity c stacks k = floor(128/c) node-columns on the partition
    axis: partition p = n_lo*c + s (n_lo in [0,k), slot s in [0,c)).  Free
    axis per 8-node block i: col = 128*i + d*8 + n8.
  - sh and cutoffs are bf16 (gate is rel_err < 2e-2; bf16 keeps us ~3e-3).
    This halves HBM traffic vs the fp32 baseline, which was HBM-bound.
  - The elementwise multiply runs on the Vector engine only (bf16 packed
    innermost -> DVE 2x mode, ~0.6 ns/col real).  GpSimd measured 2.5-5
    ns/col on this AP shape and became the critical path when given a share.
  - The segmented reduction runs on the otherwise-idle PE array: one matmul
    per 8-node block with lhsT (stationary) = scaled data [kc, 128] and
    rhs (moving) = a static ones-block [kc, k] (ones[p, j] = inv*(p//c == j),
    folding the inv scale in).  out[d*8+n8, j] lands in PSUM fp32.  This
    removes the ~30us dtype-independent DVE tensor_reduce a classic
    slot-reduction needs.
  - PSUM eviction via the Scalar (ACT) engine copy; output DMAs issue from
    the GpSimd engine (SWDGE lanes) so they never recycle one of the 8 HWDGE
    semaphores the input stream depends on.
  - DMA layout: the HW DGE only spreads a dma_start across all 16 DMA
    engines (~360 GB/s aggregate; a single engine does ~22 GB/s) when the
    transfer has ~128 rows of >= ~6.5KB.  Chunks with short rows are merged
    into shared 128-row DMA units; every input dma_start then has the
    full-spread profile and all are issued up-front.
"""

import os

import ml_dtypes
import numpy as np

# ---------------------------------------------------------------- constants
N_NODES = 50_000
D_SH = 16
N_CORES = 8
NPC = N_NODES // N_CORES          # 6250 nodes per core
NB_MAX = 28                       # max 8-node blocks per chunk (psum unit)
NB_PIECE = 14                     # blocks per Vector-multiply piece
LMIN_SOLO = 3500                  # cols; chunks with shorter rows share a unit

_NC_CACHE: dict = {}
LAST_RESULTS = None  # BassKernelResults of the most recent run (for test.py)


# ---------------------------------------------------------------- geometry
def plan_chunks(dmax: np.ndarray):
    """Greedy capacity grouping from the SPMD-uniform per-position max degree
    (descending), chunking, and DMA-unit assignment."""
    q, groups = 0, []
    npos = dmax.shape[0]
    while q < npos:
        d0 = max(int(dmax[q]), 1)
        assert d0 <= 128, f"node degree {d0} > 128 unsupported by this layout"
        k = max(1, 128 // d0)
        q2 = q
        while q2 < npos and max(1, 128 // max(int(dmax[q2]), 1)) == k:
            q2 += 1
        n = -(-(q2 - q) // (8 * k)) * (8 * k)
        n = min(n, -(-(npos - q) // (8 * k)) * (8 * k))
        c = int(dmax[q:min(q + n, npos)].max())
        groups.append((k, max(c, 1), n))
        q += n

    chunks = []
    node_start = out_off = ones_off = 0
    for k, c, n in groups:
        nb_total = n // (8 * k)
        done = 0
        while done < nb_total:
            nb = min(NB_MAX, nb_total - done)
            kc = k * c
            F = 128 * nb
            # per-partition cols of this chunk: sh data then its cutoffs
            Lc = F + 8 * nb
            chunks.append(dict(k=k, c=c, kc=kc, nb=nb, F=F, Lc=Lc,
                               node_start=node_start, out_off=out_off,
                               ones_off=ones_off))
            node_start += 8 * k * nb
            out_off += k * nb
            ones_off += k
            done += nb

    # ---- DMA units.  Small chunks (short rows) merge into shared 128-row
    # units; each >= LMIN_SOLO chunk is its own unit.  The tiny merged unit
    # also carries every chunk's ones-block columns (needed by the first
    # matmul, so it is scheduled first).
    small = [i for i, ch in enumerate(chunks) if ch["Lc"] < LMIN_SOLO]
    solo = [i for i, ch in enumerate(chunks) if ch["Lc"] >= LMIN_SOLO]
    small.sort(key=lambda i: chunks[i]["Lc"])       # tiny first
    units = []

    def add_unit(chunk_ids, with_ones):
        col = ones_off if with_ones else 0
        for ci in chunk_ids:
            chunks[ci]["unit"] = len(units)
            chunks[ci]["ucol"] = col
            col += chunks[ci]["Lc"]
        units.append(dict(chunks=list(chunk_ids), L=col,
                          with_ones=with_ones))

    if small:
        add_unit(small, True)
    else:
        add_unit([solo.pop(0)], True)
    for ci in solo:
        add_unit([ci], False)

    sh_off = 0
    for u in units:
        u["sh_off"] = sh_off
        sh_off += 128 * u["L"]

    tot = dict(nodes=node_start, sh=sh_off, out=out_off, ones=ones_off)
    # compute order: unit-0 chunks first (their data arrives first), then
    # the solo chunks in unit/DMA order
    sched = units[0]["chunks"] + [ci for u in units[1:] for ci in u["chunks"]]
    return chunks, units, tot, sched


# ---------------------------------------------------------------- device IR
def build_nc(key, chunks, units, tot, sched):
    if key in _NC_CACHE:
        return _NC_CACHE[key]

    import concourse.bacc as bacc
    import concourse.bass as bass
    import concourse.mybir as mybir
    from concourse import tile

    nc = bacc.Bacc("TRN2", target_bir_lowering=False, debug=False)

    sh = nc.dram_tensor("sh", [tot["sh"]], mybir.dt.bfloat16,
                        kind="ExternalInput")
    out = nc.dram_tensor("out", [128, tot["out"]], mybir.dt.float32,
                         kind="ExternalOutput")

    with tile.TileContext(nc) as tc:
        with (
            tc.tile_pool(name="units", bufs=1) as upool,
            tc.tile_pool(name="sclv", bufs=3) as sclvp,
            tc.tile_pool(name="osb", bufs=6) as osbp,
            tc.tile_pool(name="ps", bufs=4, space="PSUM") as psp,
        ):
            # all input DMAs issue up-front (no WAR gating), keeping every
            # DMA engine loaded and the HWDGE semaphore ring un-recycled
            utiles = []
            for u in units:
                ut = upool.tile([128, u["L"]], mybir.dt.bfloat16,
                                tag=f"u{len(utiles)}")
                src = bass.AP(sh.ap().tensor, u["sh_off"],
                              [[u["L"], 128], [1, u["L"]]])
                nc.sync.dma_start(ut[:], src)
                utiles.append(ut)

            for ci in sched:
                ch = chunks[ci]
                k, c, kc, nb, F = (ch["k"], ch["c"], ch["kc"], ch["nb"],
                                   ch["F"])
                ut = utiles[ch["unit"]]
                ubase = ut[:].offset + ch["ucol"]
                pstr = ut[:].ap[0][0]

                # scl[p, i, d, n8] = sh[p, i, d, n8] * cut[p, 8*i + n8]
                # on Vector, in NB_PIECE-block pieces so PE starts early
                scl_tiles = []
                for pi, b0 in enumerate(range(0, nb, NB_PIECE)):
                    b1 = min(nb, b0 + NB_PIECE)
                    nbe = b1 - b0
                    scl = sclvp.tile([kc, nbe * 128], mybir.dt.bfloat16,
                                     tag=f"scl{pi}")
                    sh4 = bass.AP(ut[:].tensor, ubase + b0 * 128,
                                  [[pstr, kc], [128, nbe], [8, D_SH], [1, 8]])
                    scl4 = bass.AP(scl[:].tensor, scl[:].offset,
                                   [list(scl[:].ap[0]), [128, nbe],
                                    [8, D_SH], [1, 8]])
                    cut4 = bass.AP(ut[:].tensor, ubase + F + b0 * 8,
                                   [[pstr, kc], [8, nbe], [0, D_SH], [1, 8]])
                    nc.vector.tensor_mul(scl4, sh4, cut4)
                    scl_tiles.append((scl, b0, b1))

                # PE: per 8-node block, psum[(d,n8), j] = sum_s scl[(j,s),.]
                ps_t = psp.tile([128, k * nb], mybir.dt.float32, tag="ps")
                u0 = utiles[0]
                ones_ap = bass.AP(u0[:].tensor,
                                  u0[:].offset + ch["ones_off"],
                                  [[u0[:].ap[0][0], kc], [1, k]])
                for scl, b0, b1 in scl_tiles:
                    for i in range(b0, b1):
                        lhsT = bass.AP(scl[:].tensor,
                                       scl[:].offset + (i - b0) * 128,
                                       [list(scl[:].ap[0]), [1, 128]])
                        nc.tensor.matmul(ps_t[:, i * k:(i + 1) * k],
                                         lhsT, ones_ap)

                # evict PSUM -> SBUF (inv folded into the ones weights); out
                # DMA from GpSimd's SWDGE lanes (own semaphores)
                osb = osbp.tile([128, k * nb], mybir.dt.float32, tag="osb")
                nc.scalar.copy(osb[:], ps_t[:])
                dst = bass.AP(out.ap().tensor, ch["out_off"],
                              [[tot["out"], 128], [1, k * nb]])
                nc.gpsimd.dma_start(dst, osb[:])

    nc.compile()
    _NC_CACHE[key] = nc
    return nc


# ---------------------------------------------------------------- host shard
def shard_inputs(sh_vectors, cutoffs, receivers, inv_avg_num_neighbors):
    sh_np = np.ascontiguousarray(np.asarray(sh_vectors, dtype=np.float32))
    cut_np = np.asarray(cutoffs, dtype=np.float32).ravel()
    rec = np.asarray(receivers).astype(np.int64).ravel()
    inv_val = np.float32(np.asarray(inv_avg_num_neighbors).ravel()[0])

    order = np.argsort(rec, kind="stable")
    rec_sorted = rec[order]
    first = np.searchsorted(rec_sorted, rec_sorted, side="left")
    occ = (np.arange(rec.size) - first).astype(np.int64)
    bounds = np.searchsorted(rec_sorted, np.arange(0, N_NODES + 1, NPC))

    deg = np.bincount(rec, minlength=N_NODES).reshape(N_CORES, NPC)
    node_orders, pos_of_node, dsort = [], [], np.empty((N_CORES, NPC), np.int64)
    for c in range(N_CORES):
        no = np.argsort(-deg[c], kind="stable")
        node_orders.append(no)
        pon = np.empty(NPC, dtype=np.int64)
        pon[no] = np.arange(NPC)
        pos_of_node.append(pon)
        dsort[c] = deg[c][no]
    dmax = dsort.max(axis=0)

    chunks, units, tot, sched = plan_chunks(dmax)

    # per-chunk lookups indexed by chunk id
    g_start = np.array([ch["node_start"] for ch in chunks], np.int64)
    g_k = np.array([ch["k"] for ch in chunks], np.int64)
    g_c = np.array([ch["c"] for ch in chunks], np.int64)
    g_F = np.array([ch["F"] for ch in chunks], np.int64)
    g_out = np.array([ch["out_off"] for ch in chunks], np.int64)
    # flat dram offset of (row p=0, chunk col 0) and the unit row length
    g_base = np.array([units[ch["unit"]]["sh_off"] + ch["ucol"]
                       for ch in chunks], np.int64)
    g_UL = np.array([units[ch["unit"]]["L"] for ch in chunks], np.int64)

    sh_bf = sh_np.astype(ml_dtypes.bfloat16)
    cut_bf = cut_np.astype(ml_dtypes.bfloat16)

    # template with the static ones blocks (scaled by inv) in unit 0
    template = np.zeros(tot["sh"], dtype=ml_dtypes.bfloat16)
    u0 = units[0]
    for ch in chunks:
        k, c = ch["k"], ch["c"]
        p = np.arange(k * c)
        template[u0["sh_off"] + p * u0["L"] + ch["ones_off"] + p // c] = inv_val

    in_maps = []
    decode = []
    for cid in range(N_CORES):
        lo, hi = bounds[cid], bounds[cid + 1]
        e = order[lo:hi]
        l = rec_sorted[lo:hi] - cid * NPC
        o = occ[lo:hi]
        q = pos_of_node[cid][l]

        gi = np.searchsorted(g_start, q, side="right") - 1
        k_e, c_e, F_e = g_k[gi], g_c[gi], g_F[gi]
        dq = q - g_start[gi]
        jj = dq // k_e
        n_lo = dq - jj * k_e
        i_loc = jj >> 3
        n8 = jj & 7
        p = n_lo * c_e + o
        assert (o < c_e).all(), "slot overflow: degree exceeds group capacity"

        sh_dev = template.copy()
        base = g_base[gi] + p * g_UL[gi]
        shv = sh_bf[e]
        col = 128 * i_loc + n8
        for d in range(D_SH):
            sh_dev[base + col + 8 * d] = shv[:, d]
        sh_dev[base + F_e + jj] = cut_bf[e]
        in_maps.append({"sh": sh_dev})

        # output decode for every position q in [0, NPC)
        qq = np.arange(NPC, dtype=np.int64)
        gq = np.searchsorted(g_start, qq, side="right") - 1
        dqq = qq - g_start[gq]
        jjq = dqq // g_k[gq]
        col_o = g_out[gq] + (jjq >> 3) * g_k[gq] + (dqq - jjq * g_k[gq])
        row8 = jjq & 7
        decode.append((col_o, row8))

    return in_maps, chunks, units, tot, sched, node_orders, decode


# ---------------------------------------------------------------- profiling
def _install_ntff_shim() -> bool:
    """This image's antenv lacks the axon_hooks shim that bass_utils imports
    for trace=True under axon.  Recreate it from trn_agent_boot's ctypes hook
    so NTFF profiling works.  Returns True on success."""
    try:
        import sys
        import types

        import antenv

        if getattr(antenv, "axon_hooks", None) is not None:
            return True
        import trn_agent_boot.trn_boot as tb

        hook = tb._ntff_profile_via_ctypes("/opt/axon/libaxon_pjrt.so")
        mod = types.ModuleType("antenv.axon_hooks")
        mod._hook = hook
        mod.get_axon_ntff_profile_hook = lambda: mod._hook
        mod.set_axon_ntff_profile_hook = lambda h: setattr(mod, "_hook", h)
        sys.modules["antenv.axon_hooks"] = mod
        antenv.axon_hooks = mod
        return hook is not None
    except Exception as e:  # profiling is best-effort; the run must not break
        print(f"ntff shim unavailable: {e!r}")
        return False


# ---------------------------------------------------------------- entrypoint
def kernel(sh_vectors, cutoffs, receivers, inv_avg_num_neighbors) -> np.ndarray:
    global LAST_RESULTS
    from concourse.bass_utils import run_bass_kernel_spmd

    in_maps, chunks, units, tot, sched, node_orders, decode = shard_inputs(
        sh_vectors, cutoffs, receivers, inv_avg_num_neighbors)
    key = tuple((ch["k"], ch["c"], ch["nb"], ch["unit"]) for ch in chunks)
    nc = build_nc(key, chunks, units, tot, sched)

    trace = os.environ.get("KERNEL_TRACE", "0") == "1"
    if trace:
        trace = _install_ntff_shim()
    res = run_bass_kernel_spmd(nc, in_maps, core_ids=list(range(N_CORES)),
                               trace=trace)
    LAST_RESULTS = res

    full = np.empty((N_NODES, D_SH), dtype=np.float32)
    for cid in range(N_CORES):
        o = np.asarray(res.results[cid]["out"], dtype=np.float32)
        col, row8 = decode[cid]
        blk = np.empty((NPC, D_SH), dtype=np.float32)
        for d in range(D_SH):
            blk[:, d] = o[8 * d + row8, col]
        full[cid * NPC + node_orders[cid]] = blk
    return full


# revision 35
# speedup vs baseline: 1.7389x; 1.0401x over previous
"""Trainium2 kernel for nn_EuclideanEmbedding (edge-scale + segment_sum).

Computes: out[n, :] = inv * sum_{e: receivers[e]==n} sh_vectors[e, :] * cutoffs[e]

Distribution (host side, inside kernel()):
  - Edges sharded across the 8 NeuronCores BY RECEIVER NODE RANGE: core c owns
    nodes [c*6250, (c+1)*6250) and exactly the edges targeting them.  Each core
    produces a disjoint output slice, so no collective is needed.

Device design (bf16 + PE-array segmented reduction):
  - Nodes are degree-sorted (desc) and packed into capacity groups.  A group
    with slot capacity c stacks k = floor(128/c) node-columns on the partition
    axis: partition p = n_lo*c + s (n_lo in [0,k), slot s in [0,c)).  Free
    axis per 8-node block i: col = 128*i + d*8 + n8.
  - sh and cutoffs are bf16 (gate is rel_err < 2e-2; bf16 keeps us ~3e-3).
    This halves HBM traffic vs the fp32 baseline, which was HBM-bound.
  - The elementwise multiply runs on the Vector engine only (bf16 packed
    innermost -> DVE 2x mode, ~0.6 ns/col real).  GpSimd measured 2.5-5
    ns/col on this AP shape and became the critical path when given a share.
  - The segmented reduction runs on the otherwise-idle PE array: one matmul
    per 8-node block with lhsT (stationary) = scaled data [kc, 128] and
    rhs (moving) = a static ones-block [kc, k] (ones[p, j] = inv*(p//c == j),
    folding the inv scale in).  out[d*8+n8, j] lands in PSUM fp32.  This
    removes the ~30us dtype-independent DVE tensor_reduce a classic
    slot-reduction needs.
  - PSUM eviction via the Scalar (ACT) engine copy; output DMAs issue from
    the GpSimd engine (SWDGE lanes) so they never recycle one of the 8 HWDGE
    semaphores the input stream depends on.
  - DMA layout: the HW DGE only spreads a dma_start across all 16 DMA
    engines (~360 GB/s aggregate; a single engine does ~22 GB/s) when the
    transfer has ~128 rows of >= ~6.5KB.  Chunks with short rows are merged
    into shared 128-row DMA units; every input dma_start then has the
    full-spread profile and all are issued up-front.
"""

import os

import ml_dtypes
import numpy as np

# ---------------------------------------------------------------- constants
N_NODES = 50_000
D_SH = 16
N_CORES = 8
NPC = N_NODES // N_CORES          # 6250 nodes per core
NB_MAX = 28                       # max 8-node blocks per chunk (psum unit)
NB_PIECE = 14                     # blocks per Vector-multiply piece
LMIN_SOLO = 3400                  # cols; chunks with shorter rows share a unit

_NC_CACHE: dict = {}
LAST_RESULTS = None  # BassKernelResults of the most recent run (for test.py)


# ---------------------------------------------------------------- geometry
def plan_chunks(dmax: np.ndarray):
    """Greedy capacity grouping from the SPMD-uniform per-position max degree
    (descending), chunking, and DMA-unit assignment."""
    q, groups = 0, []
    npos = dmax.shape[0]
    while q < npos:
        d0 = max(int(dmax[q]), 1)
        assert d0 <= 128, f"node degree {d0} > 128 unsupported by this layout"
        k = max(1, 128 // d0)
        q2 = q
        while q2 < npos and max(1, 128 // max(int(dmax[q2]), 1)) == k:
            q2 += 1
        n = -(-(q2 - q) // (8 * k)) * (8 * k)
        n = min(n, -(-(npos - q) // (8 * k)) * (8 * k))
        c = int(dmax[q:min(q + n, npos)].max())
        groups.append((k, max(c, 1), n))
        q += n

    chunks = []
    node_start = out_off = ones_off = 0
    for k, c, n in groups:
        nb_total = n // (8 * k)
        done = 0
        while done < nb_total:
            nb = min(NB_MAX, nb_total - done)
            kc = k * c
            F = 128 * nb
            # per-partition cols of this chunk: sh data then its cutoffs
            Lc = F + 8 * nb
            chunks.append(dict(k=k, c=c, kc=kc, nb=nb, F=F, Lc=Lc,
                               node_start=node_start, out_off=out_off,
                               ones_off=ones_off))
            node_start += 8 * k * nb
            out_off += k * nb
            ones_off += k
            done += nb

    # ---- DMA units.  Small chunks (short rows) merge into shared 128-row
    # units; each >= LMIN_SOLO chunk is its own unit.  The tiny merged unit
    # also carries every chunk's ones-block columns (needed by the first
    # matmul, so it is scheduled first).
    small = [i for i, ch in enumerate(chunks) if ch["Lc"] < LMIN_SOLO]
    solo = [i for i, ch in enumerate(chunks) if ch["Lc"] >= LMIN_SOLO]
    small.sort(key=lambda i: chunks[i]["Lc"])       # tiny first
    units = []

    def add_unit(chunk_ids, with_ones):
        col = ones_off if with_ones else 0
        for ci in chunk_ids:
            chunks[ci]["unit"] = len(units)
            chunks[ci]["ucol"] = col
            col += chunks[ci]["Lc"]
        units.append(dict(chunks=list(chunk_ids), L=col,
                          with_ones=with_ones))

    if small:
        add_unit(small, True)
    else:
        add_unit([solo.pop(0)], True)
    for ci in solo:
        add_unit([ci], False)

    sh_off = 0
    for u in units:
        u["sh_off"] = sh_off
        sh_off += 128 * u["L"]

    tot = dict(nodes=node_start, sh=sh_off, out=out_off, ones=ones_off)
    # compute order: unit-0 chunks first (their data arrives first), then
    # the solo chunks in unit/DMA order — except the smallest chunk, which
    # moves to the very end so the drain tail after the last DMA is short
    sched = units[0]["chunks"] + [ci for u in units[1:] for ci in u["chunks"]]
    tail = min(sched, key=lambda ci: chunks[ci]["Lc"])
    sched = [ci for ci in sched if ci != tail] + [tail]
    return chunks, units, tot, sched


# ---------------------------------------------------------------- device IR
def build_nc(key, chunks, units, tot, sched):
    if key in _NC_CACHE:
        return _NC_CACHE[key]

    import concourse.bacc as bacc
    import concourse.bass as bass
    import concourse.mybir as mybir
    from concourse import tile

    nc = bacc.Bacc("TRN2", target_bir_lowering=False, debug=False)

    sh = nc.dram_tensor("sh", [tot["sh"]], mybir.dt.bfloat16,
                        kind="ExternalInput")
    out = nc.dram_tensor("out", [128, tot["out"]], mybir.dt.float32,
                         kind="ExternalOutput")

    with tile.TileContext(nc) as tc:
        with (
            tc.tile_pool(name="units", bufs=1) as upool,
            tc.tile_pool(name="sclv", bufs=4) as sclvp,
            tc.tile_pool(name="osb", bufs=6) as osbp,
            tc.tile_pool(name="ps", bufs=4, space="PSUM") as psp,
        ):
            # all input DMAs issue up-front (no WAR gating), keeping every
            # DMA engine loaded and the HWDGE semaphore ring un-recycled
            utiles = []
            for u in units:
                ut = upool.tile([128, u["L"]], mybir.dt.bfloat16,
                                tag=f"u{len(utiles)}")
                src = bass.AP(sh.ap().tensor, u["sh_off"],
                              [[u["L"], 128], [1, u["L"]]])
                nc.sync.dma_start(ut[:], src)
                utiles.append(ut)

            for ci in sched:
                ch = chunks[ci]
                k, c, kc, nb, F = (ch["k"], ch["c"], ch["kc"], ch["nb"],
                                   ch["F"])
                ut = utiles[ch["unit"]]
                ubase = ut[:].offset + ch["ucol"]
                pstr = ut[:].ap[0][0]

                # scl[p, i, d, n8] = sh[p, i, d, n8] * cut[p, 8*i + n8]
                # on Vector, in NB_PIECE-block pieces so PE starts early
                scl_tiles = []
                for pi, b0 in enumerate(range(0, nb, NB_PIECE)):
                    b1 = min(nb, b0 + NB_PIECE)
                    nbe = b1 - b0
                    scl = sclvp.tile([kc, nbe * 128], mybir.dt.bfloat16,
                                     tag=f"scl{pi}")
                    sh4 = bass.AP(ut[:].tensor, ubase + b0 * 128,
                                  [[pstr, kc], [128, nbe], [8, D_SH], [1, 8]])
                    scl4 = bass.AP(scl[:].tensor, scl[:].offset,
                                   [list(scl[:].ap[0]), [128, nbe],
                                    [8, D_SH], [1, 8]])
                    cut4 = bass.AP(ut[:].tensor, ubase + F + b0 * 8,
                                   [[pstr, kc], [8, nbe], [0, D_SH], [1, 8]])
                    nc.vector.tensor_mul(scl4, sh4, cut4)
                    scl_tiles.append((scl, b0, b1))

                # PE: per 8-node block, psum[(d,n8), j] = sum_s scl[(j,s),.]
                ps_t = psp.tile([128, k * nb], mybir.dt.float32, tag="ps")
                u0 = utiles[0]
                ones_ap = bass.AP(u0[:].tensor,
                                  u0[:].offset + ch["ones_off"],
                                  [[u0[:].ap[0][0], kc], [1, k]])
                for scl, b0, b1 in scl_tiles:
                    for i in range(b0, b1):
                        lhsT = bass.AP(scl[:].tensor,
                                       scl[:].offset + (i - b0) * 128,
                                       [list(scl[:].ap[0]), [1, 128]])
                        nc.tensor.matmul(ps_t[:, i * k:(i + 1) * k],
                                         lhsT, ones_ap)

                # evict PSUM -> SBUF (inv folded into the ones weights); out
                # DMA from GpSimd's SWDGE lanes (own semaphores)
                osb = osbp.tile([128, k * nb], mybir.dt.float32, tag="osb")
                nc.scalar.copy(osb[:], ps_t[:])
                dst = bass.AP(out.ap().tensor, ch["out_off"],
                              [[tot["out"], 128], [1, k * nb]])
                nc.gpsimd.dma_start(dst, osb[:])

    nc.compile()
    _NC_CACHE[key] = nc
    return nc


# ---------------------------------------------------------------- host shard
def shard_inputs(sh_vectors, cutoffs, receivers, inv_avg_num_neighbors):
    sh_np = np.ascontiguousarray(np.asarray(sh_vectors, dtype=np.float32))
    cut_np = np.asarray(cutoffs, dtype=np.float32).ravel()
    rec = np.asarray(receivers).astype(np.int64).ravel()
    inv_val = np.float32(np.asarray(inv_avg_num_neighbors).ravel()[0])

    order = np.argsort(rec, kind="stable")
    rec_sorted = rec[order]
    first = np.searchsorted(rec_sorted, rec_sorted, side="left")
    occ = (np.arange(rec.size) - first).astype(np.int64)
    bounds = np.searchsorted(rec_sorted, np.arange(0, N_NODES + 1, NPC))

    deg = np.bincount(rec, minlength=N_NODES).reshape(N_CORES, NPC)
    node_orders, pos_of_node, dsort = [], [], np.empty((N_CORES, NPC), np.int64)
    for c in range(N_CORES):
        no = np.argsort(-deg[c], kind="stable")
        node_orders.append(no)
        pon = np.empty(NPC, dtype=np.int64)
        pon[no] = np.arange(NPC)
        pos_of_node.append(pon)
        dsort[c] = deg[c][no]
    dmax = dsort.max(axis=0)

    chunks, units, tot, sched = plan_chunks(dmax)

    # per-chunk lookups indexed by chunk id
    g_start = np.array([ch["node_start"] for ch in chunks], np.int64)
    g_k = np.array([ch["k"] for ch in chunks], np.int64)
    g_c = np.array([ch["c"] for ch in chunks], np.int64)
    g_F = np.array([ch["F"] for ch in chunks], np.int64)
    g_out = np.array([ch["out_off"] for ch in chunks], np.int64)
    # flat dram offset of (row p=0, chunk col 0) and the unit row length
    g_base = np.array([units[ch["unit"]]["sh_off"] + ch["ucol"]
                       for ch in chunks], np.int64)
    g_UL = np.array([units[ch["unit"]]["L"] for ch in chunks], np.int64)

    sh_bf = sh_np.astype(ml_dtypes.bfloat16)
    cut_bf = cut_np.astype(ml_dtypes.bfloat16)

    # template with the static ones blocks (scaled by inv) in unit 0
    template = np.zeros(tot["sh"], dtype=ml_dtypes.bfloat16)
    u0 = units[0]
    for ch in chunks:
        k, c = ch["k"], ch["c"]
        p = np.arange(k * c)
        template[u0["sh_off"] + p * u0["L"] + ch["ones_off"] + p // c] = inv_val

    in_maps = []
    decode = []
    for cid in range(N_CORES):
        lo, hi = bounds[cid], bounds[cid + 1]
        e = order[lo:hi]
        l = rec_sorted[lo:hi] - cid * NPC
        o = occ[lo:hi]
        q = pos_of_node[cid][l]

        gi = np.searchsorted(g_start, q, side="right") - 1
        k_e, c_e, F_e = g_k[gi], g_c[gi], g_F[gi]
        dq = q - g_start[gi]
        jj = dq // k_e
        n_lo = dq - jj * k_e
        i_loc = jj >> 3
        n8 = jj & 7
        p = n_lo * c_e + o
        assert (o < c_e).all(), "slot overflow: degree exceeds group capacity"

        sh_dev = template.copy()
        base = g_base[gi] + p * g_UL[gi]
        shv = sh_bf[e]
        col = 128 * i_loc + n8
        for d in range(D_SH):
            sh_dev[base + col + 8 * d] = shv[:, d]
        sh_dev[base + F_e + jj] = cut_bf[e]
        in_maps.append({"sh": sh_dev})

        # output decode for every position q in [0, NPC)
        qq = np.arange(NPC, dtype=np.int64)
        gq = np.searchsorted(g_start, qq, side="right") - 1
        dqq = qq - g_start[gq]
        jjq = dqq // g_k[gq]
        col_o = g_out[gq] + (jjq >> 3) * g_k[gq] + (dqq - jjq * g_k[gq])
        row8 = jjq & 7
        decode.append((col_o, row8))

    return in_maps, chunks, units, tot, sched, node_orders, decode


# ---------------------------------------------------------------- profiling
def _install_ntff_shim() -> bool:
    """This image's antenv lacks the axon_hooks shim that bass_utils imports
    for trace=True under axon.  Recreate it from trn_agent_boot's ctypes hook
    so NTFF profiling works.  Returns True on success."""
    try:
        import sys
        import types

        import antenv

        if getattr(antenv, "axon_hooks", None) is not None:
            return True
        import trn_agent_boot.trn_boot as tb

        hook = tb._ntff_profile_via_ctypes("/opt/axon/libaxon_pjrt.so")
        mod = types.ModuleType("antenv.axon_hooks")
        mod._hook = hook
        mod.get_axon_ntff_profile_hook = lambda: mod._hook
        mod.set_axon_ntff_profile_hook = lambda h: setattr(mod, "_hook", h)
        sys.modules["antenv.axon_hooks"] = mod
        antenv.axon_hooks = mod
        return hook is not None
    except Exception as e:  # profiling is best-effort; the run must not break
        print(f"ntff shim unavailable: {e!r}")
        return False


# ---------------------------------------------------------------- entrypoint
def kernel(sh_vectors, cutoffs, receivers, inv_avg_num_neighbors) -> np.ndarray:
    global LAST_RESULTS
    from concourse.bass_utils import run_bass_kernel_spmd

    in_maps, chunks, units, tot, sched, node_orders, decode = shard_inputs(
        sh_vectors, cutoffs, receivers, inv_avg_num_neighbors)
    key = tuple((ch["k"], ch["c"], ch["nb"], ch["unit"]) for ch in chunks)
    nc = build_nc(key, chunks, units, tot, sched)

    trace = os.environ.get("KERNEL_TRACE", "0") == "1"
    if trace:
        trace = _install_ntff_shim()
    res = run_bass_kernel_spmd(nc, in_maps, core_ids=list(range(N_CORES)),
                               trace=trace)
    LAST_RESULTS = res

    full = np.empty((N_NODES, D_SH), dtype=np.float32)
    for cid in range(N_CORES):
        o = np.asarray(res.results[cid]["out"], dtype=np.float32)
        col, row8 = decode[cid]
        blk = np.empty((NPC, D_SH), dtype=np.float32)
        for d in range(D_SH):
            blk[:, d] = o[8 * d + row8, col]
        full[cid * NPC + node_orders[cid]] = blk
    return full
